# revision 23
# baseline (speedup 1.0000x reference)
"""Trainium2 Bass kernel for nn_MAB (dense transformer block).

Reference computation (B=32, N=512, D=512, H=8, dh=64):
    q = (Q @ Wq.T + bq)  k = (K @ Wk.T + bk)  v = (K @ Wv.T + bv)
    scores = einsum("bqhd,bkhd->bhqk", q, k) / sqrt(512)
    A = softmax(scores, axis=2)            # over the QUERY axis!
    attn = einsum("bhqk,bkhd->bqhd", A, v).reshape(B, N, D)
    out = Q + attn @ Wo.T + bo
    ffn = relu(out @ W1.T + b1) @ W2.T + b2
    return out + ffn

Strategy: pure data-parallel over batch: 8 cores x 4 batches, zero
collectives.  All activations are kept TRANSPOSED on-chip ([feature,
token]) so every matmul contracts over partitions.

Speed levers vs the f32r baseline:
  * q/k/v/o projections run as fp8e4m3 DoubleRow matmuls (256-deep
    contraction per instruction, 0.5 cyc/row): weights are host-quantized
    to fp8 with power-of-two scales, Q/K are host-quantized to fp8.
  * the attention apply also runs fp8 DoubleRow: E = exp(scores) is
    written by ACT directly as fp8, and vt = v * (512/rsum) (the x512
    keeps vt out of fp8-denormal territory; the 1/512 and the fp8 weight
    scales are folded into the exp bias / eviction scales).
  * softmax-over-q runs on scores^T tiles ([key, q]): ACT exp over
    two-bank PSUM pairs; the per-key row-sums come from either the fused
    ACT accumulator (one [128,512] exp per bank) or a DVE tensor_reduce,
    split by a static knob so ACT and DVE finish together.
  * residuals are folded into the matmul accumulations (identity-matmul
    rows) so PSUM evictions are plain copies that can be placed on either
    ACT or DVE; the vt scaling runs on the otherwise idle GPSIMD engine.
  * FFN matmuls stay bf16 (fp8 there breaks the 2e-2 error budget).
"""

import math
import os
import sys

import numpy as np

sys.path.insert(0, "/opt/trn_rl_repo")

import ml_dtypes  # noqa: E402

import concourse.bass as bass  # noqa: E402
import concourse.tile as tile  # noqa: E402
from concourse import bacc  # noqa: E402
from concourse import mybir  # noqa: E402
from concourse.bass_utils import run_bass_kernel_spmd  # noqa: E402

F32 = mybir.dt.float32
F8 = mybir.dt.float8e4
BF16 = mybir.dt.bfloat16
AF = mybir.ActivationFunctionType
ALU = mybir.AluOpType
DR = mybir.MatmulPerfMode.DoubleRow

B, N, D, H = 32, 512, 512, 8
DH = D // H  # 64
NCORES = 8
BLOC = B // NCORES  # 4 batches per core
SCALE = 1.0 / math.sqrt(512.0)
P = 128
KC = D // P  # 4 contraction chunks
MC = D // P  # 4 output-feature chunks

# engine-balance knobs (tuned against TimelineSim)
N_ACC = 55          # of 64 (h,b,t) exp-pairs: how many use ACT-accum rsum
EV_ACT = 0          # of the flexible evictions: every EV_ACT-th goes to ACT

_CACHE = {}
PHASE_MARKS = []  # (instr_index, label) filled during build for trace attribution


def _build_program(with_bias):
    nc = bacc.Bacc("TRN2", target_bir_lowering=False, debug=False,
                   num_devices=NCORES)

    # ---- DRAM I/O -------------------------------------------------------
    qt8_d = nc.dram_tensor("qt8", [BLOC, D, N], F8, kind="ExternalInput").ap()
    kt8_d = nc.dram_tensor("kt8", [BLOC, D, N], F8, kind="ExternalInput").ap()
    qtb_d = nc.dram_tensor("qtb", [BLOC, D, N], BF16,
                           kind="ExternalInput").ap()
    wq_d = nc.dram_tensor("wq8", [D, D], F8, kind="ExternalInput").ap()
    wk_d = nc.dram_tensor("wk8", [D, D], F8, kind="ExternalInput").ap()
    wv_d = nc.dram_tensor("wv8", [D, D], F8, kind="ExternalInput").ap()
    wo_d = nc.dram_tensor("wo8", [DH, MC, 2, D], F8,
                          kind="ExternalInput").ap()
    w1_d = nc.dram_tensor("w1b", [D, D], BF16, kind="ExternalInput").ap()
    w2_d = nc.dram_tensor("w2b", [D, D], BF16, kind="ExternalInput").ap()
    id_d = nc.dram_tensor("idm", [P, P], BF16, kind="ExternalInput").ap()
    # cst cols: 0: exp scale  SCALE/(swq*swk); 1: exp bias ln(swv/512);
    #           2: 1/swv (attnT evict); 3: 1/swo (outT evict)
    cst_d = nc.dram_tensor("cst", [P, 4], F32, kind="ExternalInput").ap()
    b_d = {}
    if with_bias:
        # host pre-scales: bqs = swq*bq, bks = swk*bk, bvs = swv*bv,
        # bos = swo*bo, b1/b2 raw.  All as [1, D] rows.
        for nm in ("bqs", "bks", "bvs", "bos", "b1r", "b2r"):
            b_d[nm] = nc.dram_tensor(nm, [1, D], BF16,
                                     kind="ExternalInput").ap()
    outT_d = nc.dram_tensor("outT", [BLOC, D, N], BF16,
                            kind="ExternalOutput").ap()

    qt8_v = qt8_d.rearrange("b (o p) t -> b p o t", p=P)
    kt8_v = kt8_d.rearrange("b (o p) t -> b p o t", p=P)
    qtb_v = qtb_d.rearrange("b (o p) t -> b p o t", p=P)
    outT_v = outT_d.rearrange("b (o p) t -> b p o t", p=P)
    wq_v = wq_d.rearrange("(o p) n -> p o n", p=P)
    wk_v = wk_d.rearrange("(o p) n -> p o n", p=P)
    wv_v = wv_d.rearrange("(o p) n -> p o n", p=P)
    w1_v = w1_d.rearrange("(o p) n -> p o n", p=P)
    w2_v = w2_d.rearrange("(o p) n -> p o n", p=P)

    # static round-robin schedulers for the balance knobs
    acc_ctr = [0]

    def use_accum():
        i = acc_ctr[0]
        acc_ctr[0] += 1
        return (i * N_ACC) % 64 < N_ACC

    ev_ctr = [0]

    def evict_engine():
        ev_ctr[0] += 1
        if EV_ACT and ev_ctr[0] % EV_ACT == 0:
            return "act"
        return "dve"

    with tile.TileContext(nc) as tc:
        with (
            tc.tile_pool(name="wpool", bufs=1) as wpool,
            tc.tile_pool(name="qin", bufs=2) as qin_pool,
            tc.tile_pool(name="kin", bufs=2) as kin_pool,
            tc.tile_pool(name="qbin", bufs=2) as qbin_pool,
            tc.tile_pool(name="proj", bufs=2) as proj_pool,
            tc.tile_pool(name="epool", bufs=3) as e_pool,
            tc.tile_pool(name="rpool", bufs=4) as r_pool,
            tc.tile_pool(name="vtp", bufs=3) as vt_pool,
            tc.tile_pool(name="atp", bufs=2) as at_pool,
            tc.tile_pool(name="outp", bufs=2) as out_pool,
            tc.tile_pool(name="h1p", bufs=2) as h1_pool,
            tc.tile_pool(name="finp", bufs=4) as fin_pool,
            tc.tile_pool(name="psS", bufs=2, space="PSUM") as psS,
            tc.tile_pool(name="psB", bufs=1, space="PSUM") as psB,
            tc.tile_pool(name="psA", bufs=2, space="PSUM") as psA,
        ):
            # ---- resident weights / constants ---------------------------
            wq_sb = wpool.tile([P, KC, D], F8, tag="wq")
            wk_sb = wpool.tile([P, KC, D], F8, tag="wk")
            wv_sb = wpool.tile([P, KC, D], F8, tag="wv")
            wo_sb = wpool.tile([DH, MC, 2, D], F8, tag="wo")
            w1_sb = wpool.tile([P, KC, D], BF16, tag="w1")
            w2_sb = wpool.tile([P, KC, D], BF16, tag="w2")
            id_sb = wpool.tile([P, P], BF16, tag="idm")
            cst = wpool.tile([P, 4], F32, tag="cst")
            nc.sync.dma_start(out=cst[:], in_=cst_d)
            # batch-0 inputs + q/k/v weights on the HWDGE path, ordered so
            # the first projection's dependencies land first; bulk loads go
            # through the Pool SWDGE (no shared-HWDGE serialization).
            qt8_0 = qin_pool.tile([P, KC, N], F8, tag="qt8", name="qt8_0")
            kt8_0 = kin_pool.tile([P, KC, N], F8, tag="kt8", name="kt8_0")
            nc.sync.dma_start(out=qt8_0[:], in_=qt8_v[0])
            nc.sync.dma_start(out=wq_sb[:], in_=wq_v)
            nc.sync.dma_start(out=kt8_0[:], in_=kt8_v[0])
            nc.sync.dma_start(out=wk_sb[:], in_=wk_v)
            nc.sync.dma_start(out=wv_sb[:], in_=wv_v)
            nc.gpsimd.dma_start(out=id_sb[:], in_=id_d)

            b_sb = {}
            ones_sb = None
            if with_bias:
                ones_sb = wpool.tile([1, N], BF16, tag="ones")
                nc.vector.memset(ones_sb[:], 1.0)
                for nm in b_d:
                    b_sb[nm] = wpool.tile([1, D], BF16, tag=f"b_{nm}")
                    nc.sync.dma_start(out=b_sb[nm][:], in_=b_d[nm])

            exp_scale = cst[:, 0:1]
            exp_bias = cst[:, 1:2]
            at_scale = cst[0:DH, 2:3]
            out_scale = cst[:, 3:4]

            def evict_copy(dst, src, engine=None):
                if (engine or evict_engine()) == "act":
                    nc.scalar.activation(out=dst, in_=src, func=AF.Identity,
                                         scale=1.0)
                else:
                    nc.vector.tensor_copy(out=dst, in_=src)

            def evict_scale(dst, src, scale_ap):
                if evict_engine() == "act":
                    nc.scalar.activation(out=dst, in_=src, func=AF.Identity,
                                         scale=scale_ap)
                else:
                    nc.vector.tensor_scalar(out=dst, in0=src,
                                            scalar1=scale_ap, scalar2=None,
                                            op0=ALU.mult)

            def evict_relu(dst, src, engine=None):
                if (engine or evict_engine()) == "act":
                    nc.scalar.activation(out=dst, in_=src, func=AF.Relu,
                                         scale=1.0)
                else:
                    nc.vector.tensor_scalar(out=dst, in0=src, scalar1=0.0,
                                            scalar2=0.0, op0=ALU.add,
                                            op1=ALU.max)

            st = {}  # per-batch live tiles

            def emit_proj(b):
                if b == 0:
                    qt8, kt8 = qt8_0, kt8_0
                else:
                    qt8 = qin_pool.tile([P, KC, N], F8, tag="qt8")
                    kt8 = kin_pool.tile([P, KC, N], F8, tag="kt8")
                    nc.gpsimd.dma_start(out=qt8[:], in_=qt8_v[b])
                    nc.gpsimd.dma_start(out=kt8[:], in_=kt8_v[b])
                qtb = qbin_pool.tile([P, KC, N], BF16, tag="qtb")
                nc.gpsimd.dma_start(out=qtb[:], in_=qtb_v[b])

                qh = proj_pool.tile([P, MC, N], BF16, tag="qh")
                kh = proj_pool.tile([P, MC, N], BF16, tag="kh")
                vh = proj_pool.tile([P, KC, N], BF16, tag="vh")
                # batch 0 runs while ACT is still idle: split its evictions
                b0_eng = ["act", "dve"]

                def proj_evict(dst, ps):
                    if b == 0:
                        eng = b0_eng[0]
                        b0_eng.reverse()
                        evict_copy(dst, ps, engine=eng)
                    else:
                        evict_copy(dst, ps)

                def qk_chunk(dst, w_sb, rhs8, bias, m):
                    ps = psA.tile([P, N], F32, tag="psA")
                    for kp in range(2):
                        nc.tensor.matmul(
                            ps, lhsT=w_sb[:, 2 * kp:2 * kp + 2,
                                          m * P:(m + 1) * P],
                            rhs=rhs8[:, 2 * kp:2 * kp + 2, :],
                            start=(kp == 0),
                            stop=(kp == 1 and not with_bias),
                            perf_mode=DR)
                    if with_bias:
                        nc.tensor.matmul(
                            ps, lhsT=b_sb[bias][:, m * P:(m + 1) * P],
                            rhs=ones_sb[:], start=False, stop=True)
                    proj_evict(dst[:, m, :], ps)

                def v_chunk(tt):
                    # v projection (transposed [key, feat]): lhsT = K^T chunk
                    ps = psA.tile([P, N], F32, tag="psA")
                    for kp in range(2):
                        nc.tensor.matmul(
                            ps, lhsT=kt8[:, 2 * kp:2 * kp + 2,
                                         tt * P:(tt + 1) * P],
                            rhs=wv_sb[:, 2 * kp:2 * kp + 2, :],
                            start=(kp == 0),
                            stop=(kp == 1 and not with_bias),
                            perf_mode=DR)
                    if with_bias:
                        nc.tensor.matmul(ps, lhsT=ones_sb[:, 0:P],
                                         rhs=b_sb["bvs"][:],
                                         start=False, stop=True)
                    proj_evict(vh[:, tt, :], ps)

                # order chosen so attention on head-pair 0 can start early
                qk_chunk(qh, wq_sb, qt8, "bqs", 0)
                qk_chunk(kh, wk_sb, kt8, "bks", 0)
                v_chunk(0)
                v_chunk(1)
                qk_chunk(qh, wq_sb, qt8, "bqs", 1)
                qk_chunk(kh, wk_sb, kt8, "bks", 1)
                v_chunk(2)
                v_chunk(3)
                for m in (2, 3):
                    qk_chunk(qh, wq_sb, qt8, "bqs", m)
                    qk_chunk(kh, wk_sb, kt8, "bks", m)
                st[b] = {"qh": qh, "kh": kh, "vh": vh, "qtb": qtb}

            def emit_attn(b):
                qh, kh, vh = st[b]["qh"], st[b]["kh"], st[b]["vh"]
                at = at_pool.tile([DH, MC, 2, N], F8, tag="at")
                for hp in range(MC):
                    psb = psB.tile([DH, 2, N], F32, tag="psB")
                    for hh in range(2):
                        h = 2 * hp + hh
                        r0, r1 = hh * DH, (hh + 1) * DH
                        e_t = e_pool.tile([P, KC, N], F8, tag="e")
                        rs = r_pool.tile([P, KC], F32, tag="rs")
                        rr = r_pool.tile([P, KC], F32, tag="rr")
                        vt = vt_pool.tile([P, KC, DH], F8, tag="vt")
                        for t in range(2):
                            ps = psS.tile([P, 2, N], F32, tag="psS")
                            for u in range(2):
                                j = 2 * t + u
                                nc.tensor.matmul(
                                    ps[:, u, :],
                                    lhsT=kh[r0:r1, hp, j * P:(j + 1) * P],
                                    rhs=qh[r0:r1, hp, :],
                                    start=True, stop=True)
                            if use_accum():
                                for u in range(2):
                                    j = 2 * t + u
                                    nc.scalar.activation(
                                        out=e_t[:, j, :], in_=ps[:, u, :],
                                        func=AF.Exp, scale=exp_scale,
                                        bias=exp_bias,
                                        accum_out=rs[:, j:j + 1])
                            else:
                                nc.scalar.activation(
                                    out=e_t[:, 2 * t:2 * t + 2, :], in_=ps[:],
                                    func=AF.Exp, scale=exp_scale,
                                    bias=exp_bias)
                                for u in range(2):
                                    j = 2 * t + u
                                    nc.vector.tensor_reduce(
                                        out=rs[:, j:j + 1], in_=e_t[:, j, :],
                                        axis=mybir.AxisListType.X, op=ALU.add)
                        nc.vector.reciprocal(out=rr[:], in_=rs[:])
                        nc.gpsimd.tensor_tensor(
                            out=vt[:], in0=vh[:, :, h * DH:(h + 1) * DH],
                            in1=rr[:, :, None].to_broadcast((P, KC, DH)),
                            op=ALU.mult)
                        for t in range(2):
                            nc.tensor.matmul(
                                psb[:, hh, :],
                                lhsT=vt[:, 2 * t:2 * t + 2, :],
                                rhs=e_t[:, 2 * t:2 * t + 2, :],
                                start=(t == 0), stop=(t == 1),
                                perf_mode=DR)
                    evict_scale(at[:, hp, :, :], psb[:], at_scale)
                st[b]["at"] = at

            def emit_oproj(b):
                # the last batch's epilogue is the pipeline tail: split it
                # along tokens to halve the stage-barrier latency
                nts = 2 if b == BLOC - 1 else 1
                nsl = N // nts
                at, qtb = st[b]["at"], st[b]["qtb"]
                outT = out_pool.tile([P, MC, N], BF16, tag="outT")
                for m in range(MC):
                    for th in range(nts):
                        tsl = slice(th * nsl, (th + 1) * nsl)
                        ps = psA.tile([P, nsl], F32, tag="psA")
                        for hp in range(MC):
                            nc.tensor.matmul(
                                ps, lhsT=wo_sb[:, hp, :, m * P:(m + 1) * P],
                                rhs=at[:, hp, :, tsl],
                                start=(hp == 0),
                                stop=(hp == MC - 1 and not with_bias),
                                perf_mode=DR)
                        if with_bias:
                            nc.tensor.matmul(
                                ps, lhsT=b_sb["bos"][:, m * P:(m + 1) * P],
                                rhs=ones_sb[:, tsl], start=False, stop=True)
                        # outT = ps/swo + Q^T  (STT, DVE only)
                        nc.vector.scalar_tensor_tensor(
                            out=outT[:, m, tsl], in0=ps, scalar=out_scale,
                            in1=qtb[:, m, tsl], op0=ALU.mult, op1=ALU.add)
                st[b]["outT"] = outT

            def emit_ffn(b):
                nts = 2 if b == BLOC - 1 else 1
                nsl = N // nts
                tail_eng = ["act", "dve"]
                outT = st[b]["outT"]
                h1 = h1_pool.tile([P, MC, N], BF16, tag="h1")
                for m in range(MC):
                    for th in range(nts):
                        tsl = slice(th * nsl, (th + 1) * nsl)
                        ps = psA.tile([P, nsl], F32, tag="psA")
                        for kc in range(KC):
                            nc.tensor.matmul(
                                ps, lhsT=w1_sb[:, kc, m * P:(m + 1) * P],
                                rhs=outT[:, kc, tsl],
                                start=(kc == 0),
                                stop=(kc == KC - 1 and not with_bias))
                        if with_bias:
                            nc.tensor.matmul(
                                ps, lhsT=b_sb["b1r"][:, m * P:(m + 1) * P],
                                rhs=ones_sb[:, tsl], start=False, stop=True)
                        if nts == 2:
                            tail_eng.reverse()
                            evict_relu(h1[:, m, tsl], ps, engine=tail_eng[0])
                        else:
                            evict_relu(h1[:, m, tsl], ps)
                for m in range(MC):
                    for th in range(nts):
                        tsl = slice(th * nsl, (th + 1) * nsl)
                        ps = psA.tile([P, nsl], F32, tag="psA")
                        for kc in range(KC):
                            nc.tensor.matmul(
                                ps, lhsT=w2_sb[:, kc, m * P:(m + 1) * P],
                                rhs=h1[:, kc, tsl], start=(kc == 0),
                                stop=False)
                        # residual fold: psum += I.T @ outT_m
                        nc.tensor.matmul(
                            ps, lhsT=id_sb[:], rhs=outT[:, m, tsl],
                            start=False, stop=(not with_bias))
                        if with_bias:
                            nc.tensor.matmul(
                                ps, lhsT=b_sb["b2r"][:, m * P:(m + 1) * P],
                                rhs=ones_sb[:, tsl], start=False, stop=True)
                        fin = fin_pool.tile([P, nsl], BF16, tag="fin")
                        if nts == 2:
                            tail_eng.reverse()
                            evict_copy(fin[:], ps, engine=tail_eng[0])
                        else:
                            evict_copy(fin[:], ps)
                        nc.gpsimd.dma_start(out=outT_v[b][:, m, tsl],
                                            in_=fin[:])
                del st[b]

            # deferred weight loads that overlap the first projections
            def load_late_weights():
                nc.gpsimd.dma_start(out=wo_sb[:], in_=wo_d)
                nc.gpsimd.dma_start(out=w1_sb[:], in_=w1_v)
                nc.gpsimd.dma_start(out=w2_sb[:], in_=w2_v)

            def mark(label):
                # burn one instruction name to learn the current counter
                nm = nc.get_next_instruction_name()
                PHASE_MARKS.append((int(nm.split("-")[1]), label))

            # software pipeline: proj(b) || attn(b-1) || o+ffn(b-2)
            for step in range(BLOC + 2):
                if 1 <= step <= BLOC:
                    mark(f"attn{step - 1}")
                    emit_attn(step - 1)
                if step < BLOC:
                    mark(f"proj{step}")
                    emit_proj(step)
                if step == 0:
                    load_late_weights()
                if step >= 2:
                    mark(f"oproj{step - 2}")
                    emit_oproj(step - 2)
                    mark(f"ffn{step - 2}")
                    emit_ffn(step - 2)
            mark("end")

    nc.compile()
    return nc


def _pow2_scale(amax, target=64.0):
    if amax <= 0 or not np.isfinite(amax):
        return 1.0
    return float(2.0 ** round(math.log2(target / amax)))


def kernel(Q, K, Wq, bq, Wk, bk, Wv, bv, Wo, bo, W1, b1, W2, b2):
    Q = np.asarray(Q, dtype=np.float32)
    K = np.asarray(K, dtype=np.float32)
    Wq, Wk, Wv, Wo = (np.asarray(w, np.float32) for w in (Wq, Wk, Wv, Wo))
    W1, W2 = np.asarray(W1, np.float32), np.asarray(W2, np.float32)

    biases = {nm: np.asarray(v, np.float32) for nm, v in
              (("bq", bq), ("bk", bk), ("bv", bv),
               ("bo", bo), ("b1", b1), ("b2", b2))}
    with_bias = any(np.any(v) for v in biases.values())

    key = ("nc", with_bias)
    if key not in _CACHE:
        _CACHE[key] = _build_program(with_bias)
    nc = _CACHE[key]

    swq = _pow2_scale(np.abs(Wq).max())
    swk = _pow2_scale(np.abs(Wk).max())
    swv = _pow2_scale(np.abs(Wv).max())
    swo = _pow2_scale(np.abs(Wo).max())

    F8NP = ml_dtypes.float8_e4m3
    BFNP = ml_dtypes.bfloat16

    def w8T(W, s):
        return np.ascontiguousarray((W.T * s).astype(F8NP))

    # wo folded for 64-partition DoubleRow: [dh, hp, i, m]
    WoT = (Wo.T * swo).astype(F8NP)  # [feat_in, m]
    wo_f = np.ascontiguousarray(
        WoT.reshape(MC, 2, DH, D).transpose(2, 0, 1, 3)
    )  # wait: feat_in = (2hp+i)*64+p -> reshape (MC,2,DH,D) then (p,hp,i,m)

    cst = np.zeros((P, 4), np.float32)
    cst[:, 0] = SCALE / (swq * swk)
    cst[:, 1] = math.log(swv / 512.0)
    cst[:, 2] = 1.0 / swv
    cst[:, 3] = 1.0 / swo

    common = {
        "wq8": w8T(Wq, swq),
        "wk8": w8T(Wk, swk),
        "wv8": w8T(Wv, swv),
        "wo8": wo_f,
        "w1b": np.ascontiguousarray(W1.T.astype(BFNP)),
        "w2b": np.ascontiguousarray(W2.T.astype(BFNP)),
        "idm": np.eye(P, dtype=np.float32).astype(BFNP),
        "cst": cst,
    }
    if with_bias:
        common.update({
            "bqs": (biases["bq"] * swq).astype(BFNP)[None, :],
            "bks": (biases["bk"] * swk).astype(BFNP)[None, :],
            "bvs": (biases["bv"] * swv).astype(BFNP)[None, :],
            "bos": (biases["bo"] * swo).astype(BFNP)[None, :],
            "b1r": biases["b1"].astype(BFNP)[None, :],
            "b2r": biases["b2"].astype(BFNP)[None, :],
        })

    in_maps = []
    for c in range(NCORES):
        sl = slice(c * BLOC, (c + 1) * BLOC)
        qT = np.ascontiguousarray(Q[sl].transpose(0, 2, 1))
        kT = np.ascontiguousarray(K[sl].transpose(0, 2, 1))
        in_maps.append({
            "qt8": qT.astype(F8NP),
            "kt8": kT.astype(F8NP),
            "qtb": qT.astype(BFNP),
            **common,
        })

    trace = bool(int(os.environ.get("KERNEL_TRACE", "0")))
    res = run_bass_kernel_spmd(nc, in_maps, core_ids=list(range(NCORES)),
                               trace=trace)
    if trace and res.exec_time_ns is not None:
        print(f"HW exec time: {res.exec_time_ns} ns")

    out = np.empty((B, N, D), np.float32)
    for c in range(NCORES):
        out[c * BLOC:(c + 1) * BLOC] = (
            res.results[c]["outT"].astype(np.float32).transpose(0, 2, 1))
    return out


# revision 24
# speedup vs baseline: 1.0335x; 1.0335x over previous
"""Trainium2 Bass kernel for nn_MAB (dense transformer block).

Reference computation (B=32, N=512, D=512, H=8, dh=64):
    q = (Q @ Wq.T + bq)  k = (K @ Wk.T + bk)  v = (K @ Wv.T + bv)
    scores = einsum("bqhd,bkhd->bhqk", q, k) / sqrt(512)
    A = softmax(scores, axis=2)            # over the QUERY axis!
    attn = einsum("bhqk,bkhd->bqhd", A, v).reshape(B, N, D)
    out = Q + attn @ Wo.T + bo
    ffn = relu(out @ W1.T + b1) @ W2.T + b2
    return out + ffn

Strategy: pure data-parallel over batch: 8 cores x 4 batches, zero
collectives.  All activations are kept TRANSPOSED on-chip ([feature,
token]) so every matmul contracts over partitions.

Speed levers vs the f32r baseline:
  * q/k/v/o projections run as fp8e4m3 DoubleRow matmuls (256-deep
    contraction per instruction, 0.5 cyc/row): weights are host-quantized
    to fp8 with power-of-two scales, Q/K are host-quantized to fp8.
  * the attention apply also runs fp8 DoubleRow: E = exp(scores) is
    written by ACT directly as fp8, and vt = v * (512/rsum) (the x512
    keeps vt out of fp8-denormal territory; the 1/512 and the fp8 weight
    scales are folded into the exp bias / eviction scales).
  * softmax-over-q runs on scores^T tiles ([key, q]): ACT exp over
    two-bank PSUM pairs; the per-key row-sums come from either the fused
    ACT accumulator (one [128,512] exp per bank) or a DVE tensor_reduce,
    split by a static knob so ACT and DVE finish together.
  * residuals are folded into the matmul accumulations (identity-matmul
    rows) so PSUM evictions are plain copies that can be placed on either
    ACT or DVE; the vt scaling runs on the otherwise idle GPSIMD engine.
  * FFN matmuls stay bf16 (fp8 there breaks the 2e-2 error budget).
"""

import math
import os
import sys

import numpy as np

sys.path.insert(0, "/opt/trn_rl_repo")

import ml_dtypes  # noqa: E402

import concourse.bass as bass  # noqa: E402
import concourse.tile as tile  # noqa: E402
from concourse import bacc  # noqa: E402
from concourse import mybir  # noqa: E402
from concourse.bass_utils import run_bass_kernel_spmd  # noqa: E402

F32 = mybir.dt.float32
F8 = mybir.dt.float8e4
BF16 = mybir.dt.bfloat16
AF = mybir.ActivationFunctionType
ALU = mybir.AluOpType
DR = mybir.MatmulPerfMode.DoubleRow

B, N, D, H = 32, 512, 512, 8
DH = D // H  # 64
NCORES = 8
BLOC = B // NCORES  # 4 batches per core
SCALE = 1.0 / math.sqrt(512.0)
P = 128
KC = D // P  # 4 contraction chunks
MC = D // P  # 4 output-feature chunks

# engine-balance knobs (tuned against TimelineSim)
N_ACC = 55          # of 64 (h,b,t) exp-pairs: how many use ACT-accum rsum
EV_ACT = 0          # of the flexible evictions: every EV_ACT-th goes to ACT

_CACHE = {}
PHASE_MARKS = []  # (instr_index, label) filled during build for trace attribution


def _build_program(with_bias):
    nc = bacc.Bacc("TRN2", target_bir_lowering=False, debug=False,
                   num_devices=NCORES)

    # ---- DRAM I/O -------------------------------------------------------
    qt8_d = nc.dram_tensor("qt8", [BLOC, D, N], F8, kind="ExternalInput").ap()
    kt8_d = nc.dram_tensor("kt8", [BLOC, D, N], F8, kind="ExternalInput").ap()
    qtb_d = nc.dram_tensor("qtb", [BLOC, D, N], BF16,
                           kind="ExternalInput").ap()
    wq_d = nc.dram_tensor("wq8", [D, D], F8, kind="ExternalInput").ap()
    wk_d = nc.dram_tensor("wk8", [D, D], F8, kind="ExternalInput").ap()
    wv_d = nc.dram_tensor("wv8", [D, D], F8, kind="ExternalInput").ap()
    wo_d = nc.dram_tensor("wo8", [DH, MC, 2, D], F8,
                          kind="ExternalInput").ap()
    w1_d = nc.dram_tensor("w1b", [D, D], BF16, kind="ExternalInput").ap()
    w2_d = nc.dram_tensor("w2b", [D, D], BF16, kind="ExternalInput").ap()
    id_d = nc.dram_tensor("idm", [P, P], BF16, kind="ExternalInput").ap()
    # cst cols: 0: exp scale  SCALE/(swq*swk); 1: exp bias ln(swv/512);
    #           2: 1/swv (attnT evict); 3: 1/swo (outT evict)
    cst_d = nc.dram_tensor("cst", [P, 4], F32, kind="ExternalInput").ap()
    b_d = {}
    if with_bias:
        # host pre-scales: bqs = swq*bq, bks = swk*bk, bvs = swv*bv,
        # bos = swo*bo, b1/b2 raw.  All as [1, D] rows.
        for nm in ("bqs", "bks", "bvs", "bos", "b1r", "b2r"):
            b_d[nm] = nc.dram_tensor(nm, [1, D], BF16,
                                     kind="ExternalInput").ap()
    outT_d = nc.dram_tensor("outT", [BLOC, D, N], BF16,
                            kind="ExternalOutput").ap()

    qt8_v = qt8_d.rearrange("b (o p) t -> b p o t", p=P)
    kt8_v = kt8_d.rearrange("b (o p) t -> b p o t", p=P)
    qtb_v = qtb_d.rearrange("b (o p) t -> b p o t", p=P)
    outT_v = outT_d.rearrange("b (o p) t -> b p o t", p=P)
    wq_v = wq_d.rearrange("(o p) n -> p o n", p=P)
    wk_v = wk_d.rearrange("(o p) n -> p o n", p=P)
    wv_v = wv_d.rearrange("(o p) n -> p o n", p=P)
    w1_v = w1_d.rearrange("(o p) n -> p o n", p=P)
    w2_v = w2_d.rearrange("(o p) n -> p o n", p=P)

    # static round-robin schedulers for the balance knobs
    acc_ctr = [0]

    def use_accum():
        i = acc_ctr[0]
        acc_ctr[0] += 1
        return (i * N_ACC) % 64 < N_ACC

    ev_ctr = [0]

    def evict_engine():
        ev_ctr[0] += 1
        if EV_ACT and ev_ctr[0] % EV_ACT == 0:
            return "act"
        return "dve"

    with tile.TileContext(nc) as tc:
        with (
            tc.tile_pool(name="wpool", bufs=1) as wpool,
            tc.tile_pool(name="qin", bufs=2) as qin_pool,
            tc.tile_pool(name="kin", bufs=2) as kin_pool,
            tc.tile_pool(name="qbin", bufs=2) as qbin_pool,
            tc.tile_pool(name="proj", bufs=2) as proj_pool,
            tc.tile_pool(name="epool", bufs=3) as e_pool,
            tc.tile_pool(name="rpool", bufs=4) as r_pool,
            tc.tile_pool(name="vtp", bufs=3) as vt_pool,
            tc.tile_pool(name="atp", bufs=2) as at_pool,
            tc.tile_pool(name="outp", bufs=2) as out_pool,
            tc.tile_pool(name="h1p", bufs=2) as h1_pool,
            tc.tile_pool(name="finp", bufs=4) as fin_pool,
            tc.tile_pool(name="psS", bufs=2, space="PSUM") as psS,
            tc.tile_pool(name="psB", bufs=1, space="PSUM") as psB,
            tc.tile_pool(name="psA", bufs=2, space="PSUM") as psA,
        ):
            # ---- resident weights / constants ---------------------------
            wq_sb = wpool.tile([P, KC, D], F8, tag="wq")
            wk_sb = wpool.tile([P, KC, D], F8, tag="wk")
            wv_sb = wpool.tile([P, KC, D], F8, tag="wv")
            wo_sb = wpool.tile([DH, MC, 2, D], F8, tag="wo")
            w1_sb = wpool.tile([P, KC, D], BF16, tag="w1")
            w2_sb = wpool.tile([P, KC, D], BF16, tag="w2")
            id_sb = wpool.tile([P, P], BF16, tag="idm")
            cst = wpool.tile([P, 4], F32, tag="cst")
            nc.sync.dma_start(out=cst[:], in_=cst_d)
            # batch-0 inputs + q/k/v weights on the HWDGE path, ordered so
            # the first projection's dependencies land first; bulk loads go
            # through the Pool SWDGE (no shared-HWDGE serialization).
            qt8_0 = qin_pool.tile([P, KC, N], F8, tag="qt8", name="qt8_0")
            kt8_0 = kin_pool.tile([P, KC, N], F8, tag="kt8", name="kt8_0")
            nc.sync.dma_start(out=qt8_0[:], in_=qt8_v[0])
            nc.sync.dma_start(out=wq_sb[:], in_=wq_v)
            nc.sync.dma_start(out=kt8_0[:], in_=kt8_v[0])
            nc.sync.dma_start(out=wk_sb[:], in_=wk_v)
            nc.sync.dma_start(out=wv_sb[:], in_=wv_v)
            nc.gpsimd.dma_start(out=id_sb[:], in_=id_d)

            b_sb = {}
            ones_sb = None
            if with_bias:
                ones_sb = wpool.tile([1, N], BF16, tag="ones")
                nc.vector.memset(ones_sb[:], 1.0)
                for nm in b_d:
                    b_sb[nm] = wpool.tile([1, D], BF16, tag=f"b_{nm}")
                    nc.sync.dma_start(out=b_sb[nm][:], in_=b_d[nm])

            exp_scale = cst[:, 0:1]
            exp_bias = cst[:, 1:2]
            at_scale = cst[0:DH, 2:3]
            out_scale = cst[:, 3:4]

            def evict_copy(dst, src, engine=None):
                if (engine or evict_engine()) == "act":
                    nc.scalar.activation(out=dst, in_=src, func=AF.Identity,
                                         scale=1.0)
                else:
                    nc.vector.tensor_copy(out=dst, in_=src)

            def evict_scale(dst, src, scale_ap):
                if evict_engine() == "act":
                    nc.scalar.activation(out=dst, in_=src, func=AF.Identity,
                                         scale=scale_ap)
                else:
                    nc.vector.tensor_scalar(out=dst, in0=src,
                                            scalar1=scale_ap, scalar2=None,
                                            op0=ALU.mult)

            def evict_relu(dst, src, engine=None):
                if (engine or evict_engine()) == "act":
                    nc.scalar.activation(out=dst, in_=src, func=AF.Relu,
                                         scale=1.0)
                else:
                    nc.vector.tensor_scalar(out=dst, in0=src, scalar1=0.0,
                                            scalar2=0.0, op0=ALU.add,
                                            op1=ALU.max)

            st = {}  # per-batch live tiles

            def emit_proj(b):
                if b == 0:
                    qt8, kt8 = qt8_0, kt8_0
                else:
                    qt8 = qin_pool.tile([P, KC, N], F8, tag="qt8")
                    kt8 = kin_pool.tile([P, KC, N], F8, tag="kt8")
                    nc.gpsimd.dma_start(out=qt8[:], in_=qt8_v[b])
                    nc.gpsimd.dma_start(out=kt8[:], in_=kt8_v[b])
                qtb = qbin_pool.tile([P, KC, N], BF16, tag="qtb")
                nc.gpsimd.dma_start(out=qtb[:], in_=qtb_v[b])

                qh = proj_pool.tile([P, MC, N], BF16, tag="qh")
                kh = proj_pool.tile([P, MC, N], BF16, tag="kh")
                vh = proj_pool.tile([P, KC, N], BF16, tag="vh")
                # batch 0 runs while ACT is still idle: split its evictions
                b0_eng = ["act", "dve"]

                def proj_evict(dst, ps):
                    if b == 0:
                        eng = b0_eng[0]
                        b0_eng.reverse()
                        evict_copy(dst, ps, engine=eng)
                    else:
                        evict_copy(dst, ps)

                def qk_chunk(dst, w_sb, rhs8, bias, m):
                    ps = psA.tile([P, N], F32, tag="psA")
                    for kp in range(2):
                        nc.tensor.matmul(
                            ps, lhsT=w_sb[:, 2 * kp:2 * kp + 2,
                                          m * P:(m + 1) * P],
                            rhs=rhs8[:, 2 * kp:2 * kp + 2, :],
                            start=(kp == 0),
                            stop=(kp == 1 and not with_bias),
                            perf_mode=DR)
                    if with_bias:
                        nc.tensor.matmul(
                            ps, lhsT=b_sb[bias][:, m * P:(m + 1) * P],
                            rhs=ones_sb[:], start=False, stop=True)
                    proj_evict(dst[:, m, :], ps)

                def v_chunk(tt):
                    # v projection (transposed [key, feat]): lhsT = K^T chunk
                    ps = psA.tile([P, N], F32, tag="psA")
                    for kp in range(2):
                        nc.tensor.matmul(
                            ps, lhsT=kt8[:, 2 * kp:2 * kp + 2,
                                         tt * P:(tt + 1) * P],
                            rhs=wv_sb[:, 2 * kp:2 * kp + 2, :],
                            start=(kp == 0),
                            stop=(kp == 1 and not with_bias),
                            perf_mode=DR)
                    if with_bias:
                        nc.tensor.matmul(ps, lhsT=ones_sb[:, 0:P],
                                         rhs=b_sb["bvs"][:],
                                         start=False, stop=True)
                    proj_evict(vh[:, tt, :], ps)

                # order chosen so attention on head-pair 0 can start early
                qk_chunk(qh, wq_sb, qt8, "bqs", 0)
                qk_chunk(kh, wk_sb, kt8, "bks", 0)
                v_chunk(0)
                v_chunk(1)
                qk_chunk(qh, wq_sb, qt8, "bqs", 1)
                qk_chunk(kh, wk_sb, kt8, "bks", 1)
                v_chunk(2)
                v_chunk(3)
                for m in (2, 3):
                    qk_chunk(qh, wq_sb, qt8, "bqs", m)
                    qk_chunk(kh, wk_sb, kt8, "bks", m)
                st[b] = {"qh": qh, "kh": kh, "vh": vh, "qtb": qtb}

            def emit_attn(b):
                qh, kh, vh = st[b]["qh"], st[b]["kh"], st[b]["vh"]
                at = at_pool.tile([DH, MC, 2, N], F8, tag="at")
                for hp in range(MC):
                    psb = psB.tile([DH, 2, N], F32, tag="psB")
                    for hh in range(2):
                        h = 2 * hp + hh
                        r0, r1 = hh * DH, (hh + 1) * DH
                        e_t = e_pool.tile([P, KC, N], F8, tag="e")
                        rs = r_pool.tile([P, KC], F32, tag="rs")
                        rr = r_pool.tile([P, KC], F32, tag="rr")
                        vt = vt_pool.tile([P, KC, DH], F8, tag="vt")
                        for t in range(2):
                            ps = psS.tile([P, 2, N], F32, tag="psS")
                            for u in range(2):
                                j = 2 * t + u
                                nc.tensor.matmul(
                                    ps[:, u, :],
                                    lhsT=kh[r0:r1, hp, j * P:(j + 1) * P],
                                    rhs=qh[r0:r1, hp, :],
                                    start=True, stop=True)
                            if use_accum():
                                for u in range(2):
                                    j = 2 * t + u
                                    nc.scalar.activation(
                                        out=e_t[:, j, :], in_=ps[:, u, :],
                                        func=AF.Exp, scale=exp_scale,
                                        bias=exp_bias,
                                        accum_out=rs[:, j:j + 1])
                            else:
                                nc.scalar.activation(
                                    out=e_t[:, 2 * t:2 * t + 2, :], in_=ps[:],
                                    func=AF.Exp, scale=exp_scale,
                                    bias=exp_bias)
                                for u in range(2):
                                    j = 2 * t + u
                                    nc.vector.tensor_reduce(
                                        out=rs[:, j:j + 1], in_=e_t[:, j, :],
                                        axis=mybir.AxisListType.X, op=ALU.add)
                        nc.vector.reciprocal(out=rr[:], in_=rs[:])
                        nc.gpsimd.tensor_tensor(
                            out=vt[:], in0=vh[:, :, h * DH:(h + 1) * DH],
                            in1=rr[:, :, None].to_broadcast((P, KC, DH)),
                            op=ALU.mult)
                        for t in range(2):
                            nc.tensor.matmul(
                                psb[:, hh, :],
                                lhsT=vt[:, 2 * t:2 * t + 2, :],
                                rhs=e_t[:, 2 * t:2 * t + 2, :],
                                start=(t == 0), stop=(t == 1),
                                perf_mode=DR)
                    evict_scale(at[:, hp, :, :], psb[:], at_scale)
                st[b]["at"] = at

            def emit_oproj(b):
                nts = 1
                nsl = N // nts
                at, qtb = st[b]["at"], st[b]["qtb"]
                outT = out_pool.tile([P, MC, N], BF16, tag="outT")
                for m in range(MC):
                    for th in range(nts):
                        tsl = slice(th * nsl, (th + 1) * nsl)
                        ps = psA.tile([P, nsl], F32, tag="psA")
                        for hp in range(MC):
                            nc.tensor.matmul(
                                ps, lhsT=wo_sb[:, hp, :, m * P:(m + 1) * P],
                                rhs=at[:, hp, :, tsl],
                                start=(hp == 0),
                                stop=(hp == MC - 1 and not with_bias),
                                perf_mode=DR)
                        if with_bias:
                            nc.tensor.matmul(
                                ps, lhsT=b_sb["bos"][:, m * P:(m + 1) * P],
                                rhs=ones_sb[:, tsl], start=False, stop=True)
                        # outT = ps/swo + Q^T  (STT, DVE only)
                        nc.vector.scalar_tensor_tensor(
                            out=outT[:, m, tsl], in0=ps, scalar=out_scale,
                            in1=qtb[:, m, tsl], op0=ALU.mult, op1=ALU.add)
                st[b]["outT"] = outT

            def emit_ffn(b):
                nts = 1 if b < BLOC - 1 else 1
                tail_dual = (b == BLOC - 1)
                nsl = N // nts
                tail_eng = ["act", "dve"]
                outT = st[b]["outT"]
                h1 = h1_pool.tile([P, MC, N], BF16, tag="h1")
                for m in range(MC):
                    for th in range(nts):
                        tsl = slice(th * nsl, (th + 1) * nsl)
                        ps = psA.tile([P, nsl], F32, tag="psA")
                        for kc in range(KC):
                            nc.tensor.matmul(
                                ps, lhsT=w1_sb[:, kc, m * P:(m + 1) * P],
                                rhs=outT[:, kc, tsl],
                                start=(kc == 0),
                                stop=(kc == KC - 1 and not with_bias))
                        if with_bias:
                            nc.tensor.matmul(
                                ps, lhsT=b_sb["b1r"][:, m * P:(m + 1) * P],
                                rhs=ones_sb[:, tsl], start=False, stop=True)
                        if tail_dual:
                            tail_eng.reverse()
                            evict_relu(h1[:, m, tsl], ps, engine=tail_eng[0])
                        else:
                            evict_relu(h1[:, m, tsl], ps)
                for m in range(MC):
                    for th in range(nts):
                        tsl = slice(th * nsl, (th + 1) * nsl)
                        ps = psA.tile([P, nsl], F32, tag="psA")
                        for kc in range(KC):
                            nc.tensor.matmul(
                                ps, lhsT=w2_sb[:, kc, m * P:(m + 1) * P],
                                rhs=h1[:, kc, tsl], start=(kc == 0),
                                stop=False)
                        # residual fold: psum += I.T @ outT_m
                        nc.tensor.matmul(
                            ps, lhsT=id_sb[:], rhs=outT[:, m, tsl],
                            start=False, stop=(not with_bias))
                        if with_bias:
                            nc.tensor.matmul(
                                ps, lhsT=b_sb["b2r"][:, m * P:(m + 1) * P],
                                rhs=ones_sb[:, tsl], start=False, stop=True)
                        fin = fin_pool.tile([P, nsl], BF16, tag="fin")
                        if tail_dual:
                            tail_eng.reverse()
                            evict_copy(fin[:], ps, engine=tail_eng[0])
                        else:
                            evict_copy(fin[:], ps)
                        nc.gpsimd.dma_start(out=outT_v[b][:, m, tsl],
                                            in_=fin[:])
                del st[b]

            # deferred weight loads that overlap the first projections
            def load_late_weights():
                nc.gpsimd.dma_start(out=wo_sb[:], in_=wo_d)
                nc.gpsimd.dma_start(out=w1_sb[:], in_=w1_v)
                nc.gpsimd.dma_start(out=w2_sb[:], in_=w2_v)

            def mark(label):
                # burn one instruction name to learn the current counter
                nm = nc.get_next_instruction_name()
                PHASE_MARKS.append((int(nm.split("-")[1]), label))

            # software pipeline: proj(b) || attn(b-1) || o+ffn(b-2)
            for step in range(BLOC + 2):
                if 1 <= step <= BLOC:
                    mark(f"attn{step - 1}")
                    emit_attn(step - 1)
                if step < BLOC:
                    mark(f"proj{step}")
                    emit_proj(step)
                if step == 0:
                    load_late_weights()
                if step >= 2:
                    mark(f"oproj{step - 2}")
                    emit_oproj(step - 2)
                    mark(f"ffn{step - 2}")
                    emit_ffn(step - 2)
            mark("end")

    nc.compile()
    return nc


def _pow2_scale(amax, target=64.0):
    if amax <= 0 or not np.isfinite(amax):
        return 1.0
    return float(2.0 ** round(math.log2(target / amax)))


def kernel(Q, K, Wq, bq, Wk, bk, Wv, bv, Wo, bo, W1, b1, W2, b2):
    Q = np.asarray(Q, dtype=np.float32)
    K = np.asarray(K, dtype=np.float32)
    Wq, Wk, Wv, Wo = (np.asarray(w, np.float32) for w in (Wq, Wk, Wv, Wo))
    W1, W2 = np.asarray(W1, np.float32), np.asarray(W2, np.float32)

    biases = {nm: np.asarray(v, np.float32) for nm, v in
              (("bq", bq), ("bk", bk), ("bv", bv),
               ("bo", bo), ("b1", b1), ("b2", b2))}
    with_bias = any(np.any(v) for v in biases.values())

    key = ("nc", with_bias)
    if key not in _CACHE:
        _CACHE[key] = _build_program(with_bias)
    nc = _CACHE[key]

    swq = _pow2_scale(np.abs(Wq).max())
    swk = _pow2_scale(np.abs(Wk).max())
    swv = _pow2_scale(np.abs(Wv).max())
    swo = _pow2_scale(np.abs(Wo).max())

    F8NP = ml_dtypes.float8_e4m3
    BFNP = ml_dtypes.bfloat16

    def w8T(W, s):
        return np.ascontiguousarray((W.T * s).astype(F8NP))

    # wo folded for 64-partition DoubleRow: [dh, hp, i, m]
    WoT = (Wo.T * swo).astype(F8NP)  # [feat_in, m]
    wo_f = np.ascontiguousarray(
        WoT.reshape(MC, 2, DH, D).transpose(2, 0, 1, 3)
    )  # wait: feat_in = (2hp+i)*64+p -> reshape (MC,2,DH,D) then (p,hp,i,m)

    cst = np.zeros((P, 4), np.float32)
    cst[:, 0] = SCALE / (swq * swk)
    cst[:, 1] = math.log(swv / 512.0)
    cst[:, 2] = 1.0 / swv
    cst[:, 3] = 1.0 / swo

    common = {
        "wq8": w8T(Wq, swq),
        "wk8": w8T(Wk, swk),
        "wv8": w8T(Wv, swv),
        "wo8": wo_f,
        "w1b": np.ascontiguousarray(W1.T.astype(BFNP)),
        "w2b": np.ascontiguousarray(W2.T.astype(BFNP)),
        "idm": np.eye(P, dtype=np.float32).astype(BFNP),
        "cst": cst,
    }
    if with_bias:
        common.update({
            "bqs": (biases["bq"] * swq).astype(BFNP)[None, :],
            "bks": (biases["bk"] * swk).astype(BFNP)[None, :],
            "bvs": (biases["bv"] * swv).astype(BFNP)[None, :],
            "bos": (biases["bo"] * swo).astype(BFNP)[None, :],
            "b1r": biases["b1"].astype(BFNP)[None, :],
            "b2r": biases["b2"].astype(BFNP)[None, :],
        })

    in_maps = []
    for c in range(NCORES):
        sl = slice(c * BLOC, (c + 1) * BLOC)
        qT = np.ascontiguousarray(Q[sl].transpose(0, 2, 1))
        kT = np.ascontiguousarray(K[sl].transpose(0, 2, 1))
        in_maps.append({
            "qt8": qT.astype(F8NP),
            "kt8": kT.astype(F8NP),
            "qtb": qT.astype(BFNP),
            **common,
        })

    trace = bool(int(os.environ.get("KERNEL_TRACE", "0")))
    res = run_bass_kernel_spmd(nc, in_maps, core_ids=list(range(NCORES)),
                               trace=trace)
    if trace and res.exec_time_ns is not None:
        print(f"HW exec time: {res.exec_time_ns} ns")

    out = np.empty((B, N, D), np.float32)
    for c in range(NCORES):
        out[c * BLOC:(c + 1) * BLOC] = (
            res.results[c]["outT"].astype(np.float32).transpose(0, 2, 1))
    return out
